# revision 45
# baseline (speedup 1.0000x reference)
"""BigBird sparse attention kernel for Trainium2 (8 NeuronCores).

Problem (hardcoded): B=2, S=2048, H=16, D=64, block=128, G=128 global
tokens, R=64 random tokens, attn_mask is all-zeros by construction
(spec fill="zeros").

Math notes (mask == 0):
  * Diagonal branch: standard per-(b, block, head) softmax attention
    within each 128-token diagonal block.
  * Global branch: the reference contracts softmax weights only over
    their own row (einsum 'bhgs,bghd->bghd'), so the contribution is
    v[:, :G] * rowsum(softmax) == v[:, :G] (rowsum == 1 up to fp
    rounding).
  * Random branch: same structure, contribution is v[:, r] *
    rowsum(softmax) == v[:, r], scatter-added per occurrence.
  Both reduce to out[:, s] += cnt2[s] * v[:, s] with
  cnt2 = bincount(rand_indices) + (s < G), done on host.

Sharding: each of the 8 cores gets one (batch, 4-head group):
core c -> b = c // 4, heads 4*(c%4) .. 4*(c%4)+4. No collectives.

Numerics / layout (rel tolerance is 2e-2; measured ~6.5e-3):
  * q, k, v are fp8 e3m4 (4 mantissa bits, range +-15.5; inputs are
    N(0,1) so no clipping needed). Halves HBM traffic vs f16 and FWL
    quadruples the QK^T weight-load rate.
  * Scores are f32 in PSUM; exp() runs once per round over both score
    banks (ACT cost is (N+352)/1.2ns, so batching 2 blocks x 4 heads
    into one 1024-col ACT op cuts the fixed overhead 4x vs per-head).
  * Softmax weights are f16 (w_max ~200 < 65504, no bias needed).
  * PV output: per head 66 cols = [64 out | rowsum | pad]; the ones
    column in V gives the softmax denominator for free, the pad col
    keeps the PSUM->SBUF evacuation and the output DMA contiguous.
    Output stored as f16; host divides by the rowsum column.
  * Per round of 2 blocks: score tile [128, 2(sub), 512] f32 = 2 PSUM
    banks. Heads with sub=h%2==0 land in bank 0, sub=1 in bank 1, so
    the two 64-row-group QK^T matmuls of a head pair run concurrently
    on the PE without a same-bank collision. Output tile [128, 2(blk),
    512] f32 = 2 banks (264 of 512 cols used; a 4-head row of 264 f32
    cannot share a bank with another block without crossing a bank
    boundary mid-matmul). 2 score + 2 out tiles, double-buffered = 8
    banks exactly.
  * Rounds: [1] + [2]*7 + [1] blocks - the single-block first/last
    rounds shorten the DMA-gated ramp and the end-of-kernel
    exp->pv->evac->store chain.
  * Supply pacing dominates the schedule: inputs are 1.57MB and the
    exp stream consumes them at ~2 blocks/1.15us, so qk is cut into
    one chunk per round with triggers interleaved across the two HWDGE
    rings (each DIRECT2D costs ~0.6-0.75us of sequencer time and each
    chunk pays ~1.6us of trigger->usable latency; two rings halve the
    serialization). v loads ride SWDGE (gpsimd) so the ACT sequencer
    only runs the exp stream. 1-block transfers use single_packet to
    dodge the 64-packet small-DMA floor.
  * Fixed costs out of our control: ~1.4us Tile prologue, ~2.5us
    semaphore-teardown + 2 pool-release barriers, and a final
    cross-core AllGather barrier that absorbs ~2us of core launch
    skew. PE runs at 1.2 GHz throughout (HAM never sees 3.4us of
    sustained busy); warm-up dummies measured net-negative.
"""

import numpy as np

B, S, H, D = 2, 2048, 16, 64
BS = 128          # block size
NB = S // BS      # 16 diagonal blocks
G = 128           # num global tokens
SCALE = 1.0 / float(D) ** 0.5
NCORES = 8
HPC = 4           # heads per core
OD = D + 2        # 66: [64 out | rowsum | pad] per head
OROW = HPC * OD   # 264 f32/f16 cols per block
VROW = HPC * OD   # v rows match: [64 vals | 1.0 | 0.0]

# Blocks per round, in SUPPLY-COMPLETION order rather than block order:
# qk chunks alternate sync/scalar rings and each ring drains FIFO, so
# sync's 2nd chunk (blocks 7,8) completes before scalar's 2nd (5,6);
# matching round order to arrival order removes the mid-stream stall.
RBLOCKS = [[0], [1, 2], [3, 4], [7, 8], [5, 6],
           [9, 10], [11, 12], [13, 14], [15]]
RB = [len(b) for b in RBLOCKS]
NR = len(RB)

# qk chunks: one per round, triggers interleaved across the two HWDGE
# rings in need order (each trigger costs ~0.6-0.75us of sequencer
# time, so a single ring would serialize the early rounds). Only 3 on
# the ACT ring so its sequencer is free before the first exp's wait.
QCH = [1, 2, 2, 2, 2, 2, 2, 2, 1]
QOF = [0, 1, 3, 5, 7, 9, 11, 13, 15]
QENG = [0, 1, 0, 1, 0, 1, 0, 0, 0]   # 0 = sync ring, 1 = scalar ring
VCH = [2, 2, 4, 4, 4]          # v chunk sizes (blocks)
VOF = [0, 2, 4, 8, 12]
OGROUPS = [[0, 1], [2, 3], [4, 5], [6, 7], [8]]  # rounds per out chunk
# per-chunk block lists (store order) and DRAM position offsets
OBLOCKS = [sum((RBLOCKS[r] for r in g), []) for g in OGROUPS]
OCH = [len(b) for b in OBLOCKS]                  # [3, 4, 4, 4, 1]
OOF = [sum(OCH[:i]) for i in range(len(OCH))]    # position offsets

_cached = {}


def _build_program():
    import concourse.bass as bass
    import concourse.tile as tile
    from concourse import bacc, mybir

    f32 = mybir.dt.float32
    f16 = mybir.dt.float16
    f8 = mybir.dt.float8e3
    f8e4 = mybir.dt.float8e4
    AF = mybir.ActivationFunctionType

    nc = bacc.Bacc(
        "TRN2",
        target_bir_lowering=False,
        debug=False,
        enable_asserts=False,
        num_devices=NCORES,
    )
    # qk chunk layout: [128 part][pair][qk][s], partition = (h%2)*64+d,
    # pair = h//2. Chunk-tiled (QCH) so each load is one contiguous blob.
    qk = nc.dram_tensor("qk", [128 * 2 * 2 * S], f8, kind="ExternalInput").ap()
    # v chunk layout: [128 kpart][blk][h][66], col 64 = ones (softmax
    # denominator), col 65 = zeros (pad).
    v = nc.dram_tensor("v", [128 * NB * VROW], f8, kind="ExternalInput").ap()
    # out chunk layout: [128 qpart][blk][h][66] f16, unnormalized PV |
    # rowsum | pad; host divides.
    out = nc.dram_tensor("out", [128 * NB * OROW], f16, kind="ExternalOutput").ap()

    with tile.TileContext(nc) as tc:
        # Two pools only (per-tag bufs): each pool release costs an
        # all-engine barrier in the postamble, ~1.2us apiece.
        with (
            tc.tile_pool(name="p", bufs=2) as pool,
            tc.tile_pool(name="pp", bufs=2, space="PSUM") as ppool,
        ):
            # prefetch all qk chunks up front, alternating between the two
            # HWDGE rings (SP + ACT) so the chunk flow is ~2x a single
            # FIFO ring and rounds never stall on score inputs.
            qk_tiles = []  # per block: (tile, in-chunk idx)
            for ci, (off, sz) in enumerate(zip(QOF, QCH)):
                ln = sz * BS
                base = 128 * 2 * 2 * off * BS
                cnt = 128 * 2 * 2 * ln
                t = pool.tile([128, 2, 2, ln], f8, tag="qk", bufs=len(QCH),
                              name=f"qk{ci}")
                eng = nc.sync if QENG[ci] == 0 else nc.scalar
                eng.dma_start(
                    t[:],
                    qk[base : base + cnt].rearrange(
                        "(p a b s) -> p a b s", p=128, a=2, b=2
                    ),
                    # 1-block transfers otherwise pay the 64-packet floor
                    single_packet=(sz == 1),
                )
                for i in range(sz):
                    qk_tiles.append((t, i))

            # prefetch all v chunks (gpsimd/SWDGE so the ACT sequencer
            # stays clear for the exp stream)
            vmap = {}
            for off, sz in zip(VOF, VCH):
                for i in range(sz):
                    vmap[off + i] = (off, sz, i)
            v_chunk = {}
            for off, sz in zip(VOF, VCH):
                v_t = pool.tile([128, sz, HPC, OD], f8, tag="v", bufs=len(VCH),
                                name=f"v{off}")
                base = 128 * VROW * off
                nc.gpsimd.dma_start(
                    v_t[:],
                    v[base : base + 128 * VROW * sz].rearrange(
                        "(p c h d) -> p c h d", p=128, c=sz, h=HPC
                    ),
                    single_packet=(sz == 1),
                )
                v_chunk[off] = v_t

            rmap = {}  # round -> (chunk idx, chunk pos offset, size, in-chunk pos)
            for ci, g in enumerate(OGROUPS):
                pos = 0
                for r in g:
                    rmap[r] = (ci, OOF[ci], OCH[ci], pos)
                    pos += RB[r]

            state = [None] * NR
            chunk_tiles = {}



            def front(r):
                """QK^T + exp for round r"""
                st = ppool.tile([128, 2, 512], f32, tag="st", bufs=2,
                                name=f"st{r}")
                for j in range(RB[r]):
                    sb = RBLOCKS[r][j]
                    qt, sbl = qk_tiles[sb]
                    ssl = slice(sbl * BS, (sbl + 1) * BS)
                    for h in range(HPC):
                        pair, sub = divmod(h, 2)
                        dsl = slice(sub * 64, (sub + 1) * 64)
                        col = slice((j * 2 + pair) * BS, (j * 2 + pair + 1) * BS)
                        # S^T[k, q] = K'Q; sub 0/1 go to separate banks
                        # so the row-group pair runs concurrently.
                        nc.tensor.matmul(
                            st[:, sub, col],
                            lhsT=qt[dsl, pair, 1, ssl],
                            rhs=qt[dsl, pair, 0, ssl],
                            start=True, stop=True,
                        )
                w = pool.tile([128, 2, 512], f16, tag="w", bufs=3,
                              name=f"w{r}")
                ncols = RB[r] * 2 * BS
                nc.scalar.activation(
                    w[:, :, 0:ncols], st[:, :, 0:ncols], AF.Exp, scale=SCALE
                )
                state[r] = w

            def back(r):
                """PV + evacuate + store for round r"""
                w = state[r]
                o = ppool.tile([128, 2, 512], f32, tag="o", bufs=2,
                               name=f"o{r}")
                for j in range(RB[r]):
                    sb = RBLOCKS[r][j]
                    off, sz, vi = vmap[sb]
                    v_t = v_chunk[off]
                    for h in range(HPC):
                        pair, sub = divmod(h, 2)
                        col = slice((j * 2 + pair) * BS, (j * 2 + pair + 1) * BS)
                        nc.tensor.matmul(
                            o[:, j, h * OD : h * OD + OD],
                            lhsT=w[:, sub, col],
                            rhs=v_t[:, vi, h, :],
                            start=True, stop=True,
                        )
                # evacuate + store
                ci, ooff, osz, oi = rmap[r]
                if oi == 0:
                    out_t = pool.tile([128, osz, OROW], f16, tag="out", bufs=3,
                                      name=f"out{ci}")
                    chunk_tiles[ci] = out_t
                else:
                    out_t = chunk_tiles[ci]
                base = 128 * OROW * ooff
                if osz == 1:
                    # final block: two half-width evac+store chains on
                    # separate HWDGE rings - the tail's cast->trigger->
                    # flow->receipt pipeline runs twice in parallel at
                    # half size instead of once serially.
                    dview = out[base : base + 128 * OROW].rearrange(
                        "(p a d) -> p a d", p=128, a=2
                    )
                    for half, eng in ((0, nc.sync), (1, nc.scalar)):
                        cs = slice(half * (OROW // 2), (half + 1) * (OROW // 2))
                        nc.vector.tensor_copy(out_t[:, 0, cs], o[:, 0, cs])
                        eng.dma_start(
                            dview[:, half, :], out_t[:, 0, cs],
                            single_packet=True,
                        )
                else:
                    nc.vector.tensor_copy(
                        out_t[:, oi : oi + RB[r], :],
                        o[:, 0 : RB[r], 0:OROW],
                    )
                    if oi + RB[r] == osz:
                        # early chunks ride SWDGE; the next-to-last uses
                        # the SP HW ring (idle once qk drains).
                        dma_eng = nc.sync if ooff >= 11 else nc.gpsimd
                        dma_eng.dma_start(
                            out[base : base + 128 * OROW * osz].rearrange(
                                "(p c d) -> p c d", p=128, c=osz
                            ),
                            out_t[:],
                        )

            # 2-round software skew: the PE FIFO is [... QK(r+2), PV(r)
            # ...], so QK(r+2) (gated only by the st WAR on EXP(r))
            # completes inside EXP(r+1)'s duration and its semaphore is
            # already up when the ACT frees - with 1-round skew the
            # ~140ns PE-sem wake landed on the ACT critical path every
            # other round.
            for r in range(NR + 2):
                if r < NR:
                    front(r)
                if r >= 2:
                    back(r - 2)
    nc.compile()
    return nc


def _get_nc():
    if "nc" not in _cached:
        _cached["nc"] = _build_program()
    return _cached["nc"]


def _make_in_maps(q, k, v):
    import ml_dtypes

    f8 = ml_dtypes.float8_e3m4
    q = np.asarray(q, dtype=np.float32)
    k = np.asarray(k, dtype=np.float32)
    v = np.asarray(v, dtype=np.float32)

    in_maps = []
    for c in range(NCORES):
        b, hg = divmod(c, 4)
        hsl = slice(HPC * hg, HPC * (hg + 1))
        # (S, HPC, D) -> (HPC, D, S) -> pair-major [pair][64+64 part][S]
        qT = np.ascontiguousarray(q[b, :, hsl, :].transpose(1, 2, 0)).reshape(2, 128, S)
        kT = np.ascontiguousarray(k[b, :, hsl, :].transpose(1, 2, 0)).reshape(2, 128, S)
        # full[p][pair][qk][s]
        full = np.stack([qT, kT], axis=1).transpose(2, 0, 1, 3).astype(f8)
        # chunk-tile
        qkc = np.empty(128 * 2 * 2 * S, f8)
        pos = 0
        for off, sz in zip(QOF, QCH):
            ch = np.ascontiguousarray(full[:, :, :, off * BS : (off + sz) * BS])
            qkc[pos : pos + ch.size] = ch.ravel()
            pos += ch.size

        vc = v[b, :, hsl, :]  # (S, HPC, D)
        vp = np.zeros((S, HPC, OD), np.float32)
        vp[:, :, 0:D] = vc
        vp[:, :, D] = 1.0  # softmax denominator column
        vp = vp.astype(f8).reshape(NB, 128, HPC, OD)
        vflat = np.empty(128 * NB * VROW, f8)
        pos = 0
        for off, sz in zip(VOF, VCH):
            ch = np.ascontiguousarray(vp[off : off + sz].transpose(1, 0, 2, 3))
            vflat[pos : pos + ch.size] = ch.ravel()
            pos += ch.size
        in_maps.append({"qk": qkc, "v": vflat})
    return in_maps


def _unpack_out(o):
    """OCH-chunk-tiled flat f16 -> (S, HPC, OD) f32"""
    res = np.empty((NB, 128, HPC, OD), np.float32)
    o = np.asarray(o)
    pos = 0
    for blocks in OBLOCKS:
        sz = len(blocks)
        n = 128 * sz * OROW
        ch = o[pos : pos + n].astype(np.float32).reshape(128, sz, HPC, OD)
        res[blocks] = ch.transpose(1, 0, 2, 3)
        pos += n
    return res.reshape(S, HPC, OD)


def _assemble(results, v, rand_indices):
    out = np.empty((B, S, H, D), dtype=np.float32)
    for c in range(NCORES):
        b, hg = divmod(c, 4)
        o = _unpack_out(results[c]["out"])  # (S, HPC, OD): [o_unnorm | rowsum | pad]
        out[b, :, HPC * hg : HPC * (hg + 1), :] = o[:, :, 0:D] / o[:, :, D : D + 1]
    # global + random contributions: out[:, s] += cnt2[s] * v[:, s]
    ri = np.asarray(rand_indices).astype(np.int64).ravel()
    cnt = np.bincount(ri, minlength=S).astype(np.float32)
    cnt[:G] += 1.0
    nz = np.nonzero(cnt)[0]
    out[:, nz] += cnt[nz, None, None] * np.asarray(v, np.float32)[:, nz]
    return out


def _run(q, k, v, attn_mask, rand_indices, trace=False, trace_kwargs=None):
    from concourse.bass_utils import run_bass_kernel_spmd

    nc = _get_nc()
    in_maps = _make_in_maps(q, k, v)
    res = run_bass_kernel_spmd(
        nc,
        in_maps,
        list(range(NCORES)),
        trace=trace,
        **(trace_kwargs or {}),
    )
    return _assemble(res.results, v, rand_indices), res


def _reference_fallback(q, k, v, attn_mask, rand_indices):
    """Numpy replica of the reference for the (never expected per spec)
    case of a non-zero attn_mask."""
    q = np.asarray(q, np.float32)
    k = np.asarray(k, np.float32)
    v = np.asarray(v, np.float32)
    m = np.asarray(attn_mask, np.float32)
    ri = np.asarray(rand_indices).astype(np.int64).ravel()

    def softmax(x):
        x = x - x.max(axis=-1, keepdims=True)
        e = np.exp(x)
        return e / e.sum(axis=-1, keepdims=True)

    qb = q.reshape(B, NB, BS, H, D)
    kb = k.reshape(B, NB, BS, H, D)
    vb = v.reshape(B, NB, BS, H, D)
    scores = np.einsum("bnqhd,bnkhd->bnhqk", qb, kb) * SCALE
    mb = m.reshape(B, H, NB, BS, NB, BS)
    idx = np.arange(NB)
    diag = mb[:, :, idx, :, idx, :]  # (NB,B,H,BS,BS)
    scores = scores + diag.transpose(1, 0, 2, 3, 4)
    w = softmax(scores)
    out = np.einsum("bnhqk,bnkhd->bnqhd", w, vb).reshape(B, S, H, D)

    gq = q[:, :G]
    gv = v[:, :G]
    gs = np.einsum("bghd,bshd->bhgs", gq, k) * SCALE + m[:, :, :G, :]
    gw = softmax(gs)
    out[:, :G] += gv * gw.sum(axis=-1).transpose(0, 2, 1)[..., None]

    rq = q[:, ri]
    rv = v[:, ri]
    rs = np.einsum("brhd,bshd->bhrs", rq, k) * SCALE + m[:, :, ri, :]
    rw = softmax(rs)
    rowsum = rw.sum(axis=-1).transpose(0, 2, 1)  # (B,R,H)
    contrib = rv * rowsum[..., None]
    np.add.at(out, (slice(None), ri), contrib)
    return out


def kernel(q, k, v, attn_mask, rand_indices):
    am = np.asarray(attn_mask)
    if am.any():
        return _reference_fallback(q, k, v, attn_mask, rand_indices)
    out, _ = _run(q, k, v, attn_mask, rand_indices, trace=False)
    return out
